# revision 46
# baseline (speedup 1.0000x reference)
"""Trainium2 Bass kernel for AdaptiveDiffusionConv (gnn_message_passing).

Reference (per batch b):
    a   = adj * att[b]                      # [m, n]
    out = relu( x@Th0 + a^T (x@Th1 + a^T (x@Th2)) )   (Horner over K=3)

Design:
  * a = adj*att premultiplied on the HOST, shipped fp8 e4m3 (1 B/elem of
    DMA, zero on-device vector work). Feeds the PE directly: hop-1 uses
    it as the (fp8) stationary against bf16 moving v2; hop-2 uses it as
    the fp8 moving operand of a DoubleRow matmul.
  * hop-2 runs in DoubleRow perf mode (2 fp8 MACs/cell/cycle): stationary
    is w = x@Th1 + a^T v2 (cast f32->e4m3 from the hop-1 psum), moving is
    a, 256-deep contraction over m-tile pairs, psum in [ot, n] layout
    ([96, 512] banks). Output leaves transposed; host depermutes.
  * v-pass: per xt tile-chunk LDW, two 96-col matmuls (th2 -> v2 psum,
    th1 -> hop-1 bank open) — one stationary load serves both, so the
    pass is not LDWEIGHTS-bound.
  * x@Th0 opens the hop-2 banks via 4 big 512-col matmuls (th0-kron
    stationary, xt moving).
  * Theta is kron(I_6, Th_k) [96,96] in (t,f)/(t,o) order.
  * Dummy warmup matmuls run during the DMA head so the HAM activity
    monitor lifts the PE cold clock (1.2->2.4GHz) before real work.
  * Input DMA on both hw rings (sync + scalar), few big transfers (each
    dma_start costs ~570ns of ring setup).
  * relu writes res in psum-native [ot-half, n'] layout, bf16,
    contiguous; host depermutes + casts f32.

Node relabel: m = 8p + j (row tile j, partition p), n = 8q + ci; device
n-axis order is n' = ci*128 + q.

Sharding: pure data-parallel over batch B=16 across 8 cores (BL=2).
"""

import sys

sys.path.insert(0, "/opt/trn_rl_repo")

import numpy as np

import concourse.bacc as bacc
import concourse.mybir as mybir
from concourse import tile
from concourse.bass_utils import run_bass_kernel_spmd

B, N, F, T, K, O = 16, 1024, 16, 12, 3, 16
NCORES = 8
BL = B // NCORES  # 2 batches per core
P = 128
NT = N // P  # 8 node tiles
OT = O * T  # 192 cols per tile, (c,t',o) order
HC = 96  # contraction chunk rows (t in 0..5 | 6..11, f); theta block size
KHC = K * HC

F32 = mybir.dt.float32
BF16 = mybir.dt.bfloat16
FP8 = mybir.dt.float8e4  # e4m3 — required for DoubleRow operands
NP_BF16 = mybir.dt.np(BF16)
NP_FP8 = mybir.dt.np(FP8)
DR = mybir.MatmulPerfMode.DoubleRow

WARMUP_MMS = 15  # dummy matmuls bridging the DMA head (HAM warmup)

_CACHE = {}


def build_nc():
    nc = bacc.Bacc()

    a_ext = nc.declare_dram_parameter("a", [BL, NT, P, N], FP8, isOutput=False)
    xth_ext = nc.declare_dram_parameter(
        "xth", [HC, KHC + BL * 2 * N], BF16, isOutput=False
    )
    out_ext = nc.declare_dram_parameter("out", [BL, 2, HC, N], BF16, isOutput=True)

    with tile.TileContext(nc) as tc:
        with (
            tc.tile_pool(name="big", bufs=1) as big,
            tc.tile_pool(name="psp", bufs=8, space="PSUM") as psp,
        ):
            a_sb = big.tile([P, BL * NT * N], FP8)  # [p, (b, j, n')]
            xth_sb = big.tile([HC, KHC + BL * 2 * N], BF16)
            th_sb = xth_sb[:, :KHC]  # [th2 | th1 | th0]
            xt_sb = xth_sb[:, KHC:]  # cols (b, i, c, q)
            vw = big.tile([P, BL * NT * OT], BF16)  # v2 per (b, tile)
            w_sb = big.tile([P, BL * NT * OT], FP8)  # w per (b, tile)
            res = big.tile([HC, BL * 2 * N], BF16)  # [ot-half, (b, h, n')]
            wu_sb = big.tile([P, 256], BF16)  # warmup operand (zeros)

            a_v = a_sb.rearrange("p (b j n) -> p b j n", b=BL, j=NT)
            w_v = w_sb.rearrange("p (b j f) -> p b j f", b=BL, j=NT)
            xt_v = xt_sb.rearrange("r (b i c q) -> r b i c q", b=BL, i=NT, c=2)

            # ---- PE warmup
            nc.gpsimd.memset(wu_sb[:], 0.0)
            wu_ps = psp.tile([P, 512], F32, tag="ps")
            for _ in range(WARMUP_MMS):
                nc.tensor.matmul(
                    wu_ps[:, :256], wu_sb[:, :128], wu_sb[:], start=True, stop=True
                )

            # ---- input DMA on BOTH hw rings, few big transfers.
            # sync ring: th+xt(b0), a(b0) halves, outs. scalar: xt(b1), a(b1).
            # need-ordered across both rings: b0's data leads on each ring,
            # b0's a in j-pair quarters so hop-1 visits chase arrivals
            def dma_a(eng, b, j0, nj):
                lo = (b * NT + j0) * N
                hi = (b * NT + j0 + nj) * N
                eng.dma_start(
                    a_sb[:, lo:hi].rearrange("p (j n) -> p j n", j=nj),
                    a_ext[b, j0 : j0 + nj].rearrange("j p n -> p j n"),
                )

            # sync ring leads with th+xt(b0) in two pieces (gates the
            # v-pass pair by pair), scalar ring leads with a(b0,j0-3)
            # (gates hop-1 visit 1).
            mid = KHC + N  # th + xt(b0, tiles 0-3)
            m2 = KHC + 2 * N
            nc.sync.dma_start(xth_sb[:, :mid], xth_ext[:, :mid])
            nc.scalar.dma_start(xth_sb[:, mid:m2], xth_ext[:, mid:m2])
            dma_a(nc.scalar, 0, 0, 4)
            dma_a(nc.sync, 0, 4, 4)
            nc.scalar.dma_start(xth_sb[:, m2:], xth_ext[:, m2:])
            dma_a(nc.sync, 1, 0, 4)
            dma_a(nc.scalar, 1, 4, 4)

            def a_sl(b, ci, j):
                # [p, q] = a[8p+j, 8q+ci] (stationary block for hop-1)
                base = (b * NT + j) * N
                return a_sb[:, base + ci * P : base + (ci + 1) * P]

            def xt_sl(b, i, c):
                base = ((b * NT + i) * 2 + c) * P
                return xt_sb[:, base : base + P]

            def v2_sl(b, j):
                return vw[:, (b * NT + j) * OT : (b * NT + j + 1) * OT]

            def w_sl(b, j):
                return w_sb[:, (b * NT + j) * OT : (b * NT + j + 1) * OT]

            def stream(b):
                # psum: 4 v2-pair banks + 4 hop-1 pair banks ([i | i+1]),
                # allocated so hop-2 banks later land on the v2 slots
                # (which free early) in the pool's round-robin.
                v2ps, h1ps = [], []
                for k in range(4):
                    t = psp.tile([P, 2 * OT], F32, tag="ps", name=f"v2ps{b}_{k}")
                    v2ps.append(t)
                    if k % 2 == 1:
                        for r in range(2):
                            t = psp.tile(
                                [P, 2 * OT], F32, tag="ps", name=f"h1ps{b}_{k}_{r}"
                            )
                            h1ps.append(t)

                # v-pass: one xt LDW serves th2 (v2) + th1 (hop-1 open)
                for i in range(NT):
                    for c in range(2):
                        # one start=True per 2KB bank (it marks the whole
                        # bank pending-zero; later first-touches overwrite)
                        nc.tensor.matmul(
                            v2ps[i // 2][:, ((i % 2) * 2 + c) * HC :][:, :HC],
                            xt_sl(b, i, c),
                            th_sb[:, :HC],
                            start=(i % 2 == 0 and c == 0),
                            stop=(i % 2 == 1 and c == 1),
                            skip_group_check=True,
                        )
                        nc.tensor.matmul(
                            h1ps[i // 2][:, (i % 2) * OT + c * HC :][:, :HC],
                            xt_sl(b, i, c),
                            th_sb[:, HC : 2 * HC],
                            start=(i % 2 == 0 and c == 0), stop=False,
                            skip_group_check=True,
                        )
                    if i % 2 == 1:
                        nc.scalar.copy(
                            vw[:, (b * NT + i - 1) * OT : (b * NT + i + 1) * OT],
                            v2ps[i // 2][:],
                        )

                h2ps = []

                def open_h2():
                    # hop-2 banks [96, 512]: x@Th0 in [ot, n'] layout
                    for h in range(2):
                        for g in range(2):
                            ps = psp.tile(
                                [HC, 512], F32, tag="ps", name=f"h2ps{b}_{h}_{g}"
                            )
                            h2ps.append(ps)
                            nc.tensor.matmul(
                                ps[:],
                                th_sb[:, 2 * HC :],
                                xt_v[:, b, 4 * g : 4 * g + 4, h, :],
                                start=True, stop=False,
                                skip_group_check=True,
                            )

                # hop 1 in j-group visits (matching the a-DMA halves) so
                # the PE chases the stream instead of waiting for all of a
                nvis = 2
                jper = NT // nvis
                for v in range(nvis - 1):
                    for ci in range(NT):
                        dst = h1ps[ci // 2][:, (ci % 2) * OT :][:, :OT]
                        for j in range(v * jper, (v + 1) * jper):
                            nc.tensor.matmul(
                                dst, a_sl(b, ci, j), v2_sl(b, j),
                                start=False, stop=False,
                                skip_group_check=True,
                            )
                for ci in range(NT):
                    dst = h1ps[ci // 2][:, (ci % 2) * OT :][:, :OT]
                    for j in range(NT - jper, NT):
                        nc.tensor.matmul(
                            dst, a_sl(b, ci, j), v2_sl(b, j),
                            start=False, stop=(j == NT - 1),
                            skip_group_check=True,
                        )
                    # even copies on DVE, odd on ACT: a pair's two w-copies
                    # run concurrently, unblocking its DR step sooner
                    if ci % 2 == 0:
                        nc.vector.tensor_copy(w_sl(b, ci)[:], dst)
                    else:
                        nc.scalar.copy(w_sl(b, ci)[:], dst)
                    if b == 1 and ci == 5:
                        # prewarm both out rings (~3KB re-sends of already-
                        # final b0 data) so the final out-DMAs skip the
                        # ring-restart trickle
                        nc.sync.dma_start(
                            out_ext[0, 1][:, :16], res[:, N : N + 16]
                        )
                        nc.scalar.dma_start(
                            out_ext[0, 0][:, :16], res[:, 0:16]
                        )
                    if ci % 2 == 1:
                        u = ci // 2
                        if u == 0:
                            open_h2()
                        # h=1 first on the final step so its relu+out-DMA
                        # overlaps h=0's last matmuls
                        for h in (1, 0) if u == 3 else (0, 1):
                            for g in range(2):
                                nc.tensor.matmul(
                                    h2ps[h * 2 + g][:],
                                    w_v[:, b, ci - 1 : ci + 1, h * HC : (h + 1) * HC],
                                    a_v[:, b, ci - 1 : ci + 1, g * 512 : (g + 1) * 512],
                                    start=False, stop=(u == 3),
                                    skip_group_check=True,
                                    perf_mode=DR,
                                )
                return h2ps

            def relus(b, h2ps):
                # h1 finishes first (step-3 matmul order): its out-DMA on
                # sync overlaps h0's relus; h0's goes out on the scalar
                # ring, issued by ACT right after its own relu
                for h in (1, 0):
                    for g in range(2):
                        ps = h2ps[h * 2 + g]
                        dst = res[:, (b * 2 + h) * N + g * 512 :][:, :512]
                        if g == 0:
                            nc.vector.tensor_scalar_max(dst, ps[:], 0.0)
                        else:
                            nc.scalar.activation(
                                dst, ps[:], mybir.ActivationFunctionType.Relu
                            )
                    eng = nc.sync if h == 1 else nc.scalar
                    eng.dma_start(
                        out_ext[b, h], res[:, (b * 2 + h) * N :][:, :N]
                    )

            h2_0 = stream(0)
            relus(0, h2_0)
            h2_1 = stream(1)
            relus(1, h2_1)

    nc.compile()
    return nc


def make_in_maps(x, att, adj, Theta):
    """Host prep: a=adj*att premultiply, fp8/bf16 casts, layout permutes."""
    x = np.asarray(x, np.float32)
    att = np.asarray(att, np.float32)
    adj = np.asarray(adj, np.float32)
    Theta = np.asarray(Theta, np.float32)

    # a[b] = adj * att[b] -> [b, j, p, (ci, q)]: m=8p+j rows, n'=ci*128+q
    a_full = (adj[None, :, :] * att).astype(NP_FP8)  # [B, m, n]
    a6 = a_full.reshape(B, P, NT, P, NT)  # [b, p, j, q, ci]
    a_dev = np.ascontiguousarray(a6.transpose(0, 2, 1, 4, 3)).reshape(B, NT, P, N)

    # xt: [(t',f) rows, (b, i, c, q) cols], n = 8q+i
    xq = x.reshape(B, P, NT, F, T)  # [b, q, i, f, t]
    xt = xq.transpose(0, 2, 4, 3, 1)  # [b, i, t, f, q]
    xt = xt.reshape(B, NT, 2, 6, F, P).reshape(B, NT, 2, 6 * F, P)
    xt = np.ascontiguousarray(xt.transpose(3, 0, 1, 2, 4))  # [96, b, i, c, q]
    xt = xt.reshape(HC, B, 2 * N).astype(NP_BF16)

    th_dev = np.zeros((HC, KHC), np.float32)
    eye6 = np.eye(6, dtype=np.float32)
    for k in range(K):  # stored order [th2 | th1 | th0]
        th_dev[:, (K - 1 - k) * HC : (K - k) * HC] = np.kron(eye6, Theta[k])
    th_dev = th_dev.astype(NP_BF16)

    in_maps = []
    for c0 in range(NCORES):
        b0 = BL * c0
        xth = np.empty((HC, KHC + BL * 2 * N), NP_BF16)
        xth[:, :KHC] = th_dev
        xth[:, KHC:] = xt[:, b0 : b0 + BL].reshape(HC, -1)
        in_maps.append(
            {
                "a": np.ascontiguousarray(a_dev[b0 : b0 + BL]),
                "xth": xth,
            }
        )
    return in_maps


def depermute_out(dev_out):
    """Device out [BL, h, (t',o), n'=(ci,q)] -> [BL, n=8q+ci, o, t=6h+t']."""
    o = np.asarray(dev_out).astype(np.float32)
    o = o.reshape(BL, 2, 6, O, NT, P).transpose(0, 5, 4, 3, 1, 2)
    return np.ascontiguousarray(o.reshape(BL, N, O, T))


def kernel(x, spatial_attention, adj, Theta):
    if "nc" not in _CACHE:
        _CACHE["nc"] = build_nc()
    nc = _CACHE["nc"]

    in_maps = make_in_maps(x, spatial_attention, adj, Theta)
    res = run_bass_kernel_spmd(nc, in_maps, core_ids=list(range(NCORES)))
    return np.concatenate(
        [depermute_out(res.results[c]["out"]) for c in range(NCORES)], axis=0
    )


# revision 47
# speedup vs baseline: 1.0759x; 1.0759x over previous
"""Trainium2 Bass kernel for AdaptiveDiffusionConv (gnn_message_passing).

Reference (per batch b):
    a   = adj * att[b]                      # [m, n]
    out = relu( x@Th0 + a^T (x@Th1 + a^T (x@Th2)) )   (Horner over K=3)

Design:
  * a = adj*att premultiplied on the HOST, shipped fp8 e4m3 (1 B/elem of
    DMA, zero on-device vector work). Feeds the PE directly: hop-1 uses
    it as the (fp8) stationary against bf16 moving v2; hop-2 uses it as
    the fp8 moving operand of a DoubleRow matmul.
  * hop-2 runs in DoubleRow perf mode (2 fp8 MACs/cell/cycle): stationary
    is w = x@Th1 + a^T v2 (cast f32->e4m3 from the hop-1 psum), moving is
    a, 256-deep contraction over m-tile pairs, psum in [ot, n] layout
    ([96, 512] banks). Output leaves transposed; host depermutes.
  * v-pass: per xt tile-chunk LDW, two 96-col matmuls (th2 -> v2 psum,
    th1 -> hop-1 bank open) — one stationary load serves both, so the
    pass is not LDWEIGHTS-bound.
  * x@Th0 opens the hop-2 banks via 4 big 512-col matmuls (th0-kron
    stationary, xt moving).
  * Theta is kron(I_6, Th_k) [96,96] in (t,f)/(t,o) order.
  * Dummy warmup matmuls run during the DMA head so the HAM activity
    monitor lifts the PE cold clock (1.2->2.4GHz) before real work.
  * Input DMA on both hw rings (sync + scalar), few big transfers (each
    dma_start costs ~570ns of ring setup).
  * relu writes res in psum-native [ot-half, n'] layout, bf16,
    contiguous; host depermutes + casts f32.

Node relabel: m = 8p + j (row tile j, partition p), n = 8q + ci; device
n-axis order is n' = ci*128 + q.

Sharding: pure data-parallel over batch B=16 across 8 cores (BL=2).
"""

import sys

sys.path.insert(0, "/opt/trn_rl_repo")

import numpy as np

import concourse.bacc as bacc
import concourse.mybir as mybir
from concourse import tile
from concourse.bass_utils import run_bass_kernel_spmd

B, N, F, T, K, O = 16, 1024, 16, 12, 3, 16
NCORES = 8
BL = B // NCORES  # 2 batches per core
P = 128
NT = N // P  # 8 node tiles
OT = O * T  # 192 cols per tile, (c,t',o) order
HC = 96  # contraction chunk rows (t in 0..5 | 6..11, f); theta block size
KHC = K * HC

F32 = mybir.dt.float32
BF16 = mybir.dt.bfloat16
FP8 = mybir.dt.float8e4  # e4m3 — required for DoubleRow operands
NP_BF16 = mybir.dt.np(BF16)
NP_FP8 = mybir.dt.np(FP8)
DR = mybir.MatmulPerfMode.DoubleRow

WARMUP_MMS = 15  # dummy matmuls bridging the DMA head (HAM warmup)

_CACHE = {}


def build_nc():
    nc = bacc.Bacc()

    a_ext = nc.declare_dram_parameter("a", [BL, NT, P, N], FP8, isOutput=False)
    xth_ext = nc.declare_dram_parameter(
        "xth", [HC, KHC + BL * 2 * N], BF16, isOutput=False
    )
    out_ext = nc.declare_dram_parameter("out", [BL, 2, HC, N], BF16, isOutput=True)

    with tile.TileContext(nc) as tc:
        with (
            tc.tile_pool(name="big", bufs=1) as big,
            tc.tile_pool(name="psp", bufs=8, space="PSUM") as psp,
        ):
            a_sb = big.tile([P, BL * NT * N], FP8)  # [p, (b, j, n')]
            xth_sb = big.tile([HC, KHC + BL * 2 * N], BF16)
            th_sb = xth_sb[:, :KHC]  # [th2 | th1 | th0]
            xt_sb = xth_sb[:, KHC:]  # cols (b, i, c, q)
            vw = big.tile([P, BL * NT * OT], BF16)  # v2 per (b, tile)
            w_sb = big.tile([P, BL * NT * OT], FP8)  # w per (b, tile)
            res = big.tile([HC, BL * 2 * N], BF16)  # [ot-half, (b, h, n')]
            wu_sb = big.tile([P, 256], BF16)  # warmup operand (zeros)

            a_v = a_sb.rearrange("p (b j n) -> p b j n", b=BL, j=NT)
            w_v = w_sb.rearrange("p (b j f) -> p b j f", b=BL, j=NT)
            xt_v = xt_sb.rearrange("r (b i c q) -> r b i c q", b=BL, i=NT, c=2)

            # ---- PE warmup
            nc.gpsimd.memset(wu_sb[:], 0.0)
            wu_ps = psp.tile([P, 512], F32, tag="ps")
            for _ in range(WARMUP_MMS):
                nc.tensor.matmul(
                    wu_ps[:, :256], wu_sb[:, :128], wu_sb[:], start=True, stop=True
                )

            # ---- input DMA on BOTH hw rings, few big transfers.
            # sync ring: th+xt(b0), a(b0) halves, outs. scalar: xt(b1), a(b1).
            # need-ordered across both rings: b0's data leads on each ring,
            # b0's a in j-pair quarters so hop-1 visits chase arrivals
            def dma_a(eng, b, j0, nj):
                lo = (b * NT + j0) * N
                hi = (b * NT + j0 + nj) * N
                eng.dma_start(
                    a_sb[:, lo:hi].rearrange("p (j n) -> p j n", j=nj),
                    a_ext[b, j0 : j0 + nj].rearrange("j p n -> p j n"),
                )

            # sync ring leads with th+xt(b0) in two pieces (gates the
            # v-pass pair by pair), scalar ring leads with a(b0,j0-3)
            # (gates hop-1 visit 1).
            mid = KHC + N  # th + xt(b0, tiles 0-3)
            m2 = KHC + 2 * N
            nc.sync.dma_start(xth_sb[:, :mid], xth_ext[:, :mid])
            dma_a(nc.scalar, 0, 0, 4)
            nc.sync.dma_start(xth_sb[:, mid:m2], xth_ext[:, mid:m2])
            nc.scalar.dma_start(xth_sb[:, m2:], xth_ext[:, m2:])
            dma_a(nc.sync, 0, 4, 4)
            dma_a(nc.sync, 1, 0, 4)
            dma_a(nc.scalar, 1, 4, 4)

            def a_sl(b, ci, j):
                # [p, q] = a[8p+j, 8q+ci] (stationary block for hop-1)
                base = (b * NT + j) * N
                return a_sb[:, base + ci * P : base + (ci + 1) * P]

            def xt_sl(b, i, c):
                base = ((b * NT + i) * 2 + c) * P
                return xt_sb[:, base : base + P]

            def v2_sl(b, j):
                return vw[:, (b * NT + j) * OT : (b * NT + j + 1) * OT]

            def w_sl(b, j):
                return w_sb[:, (b * NT + j) * OT : (b * NT + j + 1) * OT]

            def stream(b):
                # psum: 4 v2-pair banks + 4 hop-1 pair banks ([i | i+1]),
                # allocated so hop-2 banks later land on the v2 slots
                # (which free early) in the pool's round-robin.
                v2ps, h1ps = [], []
                for k in range(4):
                    t = psp.tile([P, 2 * OT], F32, tag="ps", name=f"v2ps{b}_{k}")
                    v2ps.append(t)
                    if k % 2 == 1:
                        for r in range(2):
                            t = psp.tile(
                                [P, 2 * OT], F32, tag="ps", name=f"h1ps{b}_{k}_{r}"
                            )
                            h1ps.append(t)

                # v-pass: one xt LDW serves th2 (v2) + th1 (hop-1 open)
                for i in range(NT):
                    for c in range(2):
                        # one start=True per 2KB bank (it marks the whole
                        # bank pending-zero; later first-touches overwrite)
                        nc.tensor.matmul(
                            v2ps[i // 2][:, ((i % 2) * 2 + c) * HC :][:, :HC],
                            xt_sl(b, i, c),
                            th_sb[:, :HC],
                            start=(i % 2 == 0 and c == 0),
                            stop=(i % 2 == 1 and c == 1),
                            skip_group_check=True,
                        )
                        nc.tensor.matmul(
                            h1ps[i // 2][:, (i % 2) * OT + c * HC :][:, :HC],
                            xt_sl(b, i, c),
                            th_sb[:, HC : 2 * HC],
                            start=(i % 2 == 0 and c == 0), stop=False,
                            skip_group_check=True,
                        )
                    if i % 2 == 1:
                        nc.scalar.copy(
                            vw[:, (b * NT + i - 1) * OT : (b * NT + i + 1) * OT],
                            v2ps[i // 2][:],
                        )

                h2ps = []

                def open_h2():
                    # hop-2 banks [96, 512]: x@Th0 in [ot, n'] layout
                    for h in range(2):
                        for g in range(2):
                            ps = psp.tile(
                                [HC, 512], F32, tag="ps", name=f"h2ps{b}_{h}_{g}"
                            )
                            h2ps.append(ps)
                            nc.tensor.matmul(
                                ps[:],
                                th_sb[:, 2 * HC :],
                                xt_v[:, b, 4 * g : 4 * g + 4, h, :],
                                start=True, stop=False,
                                skip_group_check=True,
                            )

                # hop 1 in j-group visits (matching the a-DMA halves) so
                # the PE chases the stream instead of waiting for all of a
                nvis = 2
                jper = NT // nvis
                for v in range(nvis - 1):
                    for ci in range(NT):
                        dst = h1ps[ci // 2][:, (ci % 2) * OT :][:, :OT]
                        for j in range(v * jper, (v + 1) * jper):
                            nc.tensor.matmul(
                                dst, a_sl(b, ci, j), v2_sl(b, j),
                                start=False, stop=False,
                                skip_group_check=True,
                            )
                for ci in range(NT):
                    dst = h1ps[ci // 2][:, (ci % 2) * OT :][:, :OT]
                    for j in range(NT - jper, NT):
                        nc.tensor.matmul(
                            dst, a_sl(b, ci, j), v2_sl(b, j),
                            start=False, stop=(j == NT - 1),
                            skip_group_check=True,
                        )
                    # even copies on DVE, odd on ACT: a pair's two w-copies
                    # run concurrently, unblocking its DR step sooner
                    if ci % 2 == 0:
                        nc.vector.tensor_copy(w_sl(b, ci)[:], dst)
                    else:
                        nc.scalar.copy(w_sl(b, ci)[:], dst)
                    if b == 1 and ci == 5:
                        # prewarm both out rings (~3KB re-sends of already-
                        # final b0 data) so the final out-DMAs skip the
                        # ring-restart trickle
                        nc.sync.dma_start(
                            out_ext[0, 1][:, :16], res[:, N : N + 16]
                        )
                        nc.scalar.dma_start(
                            out_ext[0, 0][:, :16], res[:, 0:16]
                        )
                    if ci % 2 == 1:
                        u = ci // 2
                        if u == 0:
                            open_h2()
                        # h=1 first on the final step so its relu+out-DMA
                        # overlaps h=0's last matmuls
                        for h in (1, 0) if u == 3 else (0, 1):
                            for g in range(2):
                                nc.tensor.matmul(
                                    h2ps[h * 2 + g][:],
                                    w_v[:, b, ci - 1 : ci + 1, h * HC : (h + 1) * HC],
                                    a_v[:, b, ci - 1 : ci + 1, g * 512 : (g + 1) * 512],
                                    start=False, stop=(u == 3),
                                    skip_group_check=True,
                                    perf_mode=DR,
                                )
                return h2ps

            def relus(b, h2ps):
                # h1 finishes first (step-3 matmul order): its out-DMA on
                # sync overlaps h0's relus; h0's goes out on the scalar
                # ring, issued by ACT right after its own relu
                for h in (1, 0):
                    for g in range(2):
                        ps = h2ps[h * 2 + g]
                        dst = res[:, (b * 2 + h) * N + g * 512 :][:, :512]
                        if g == 0:
                            nc.vector.tensor_scalar_max(dst, ps[:], 0.0)
                        else:
                            nc.scalar.activation(
                                dst, ps[:], mybir.ActivationFunctionType.Relu
                            )
                    eng = nc.sync if h == 1 else nc.scalar
                    eng.dma_start(
                        out_ext[b, h], res[:, (b * 2 + h) * N :][:, :N]
                    )

            h2_0 = stream(0)
            relus(0, h2_0)
            h2_1 = stream(1)
            relus(1, h2_1)

    nc.compile()
    return nc


def make_in_maps(x, att, adj, Theta):
    """Host prep: a=adj*att premultiply, fp8/bf16 casts, layout permutes."""
    x = np.asarray(x, np.float32)
    att = np.asarray(att, np.float32)
    adj = np.asarray(adj, np.float32)
    Theta = np.asarray(Theta, np.float32)

    # a[b] = adj * att[b] -> [b, j, p, (ci, q)]: m=8p+j rows, n'=ci*128+q
    a_full = (adj[None, :, :] * att).astype(NP_FP8)  # [B, m, n]
    a6 = a_full.reshape(B, P, NT, P, NT)  # [b, p, j, q, ci]
    a_dev = np.ascontiguousarray(a6.transpose(0, 2, 1, 4, 3)).reshape(B, NT, P, N)

    # xt: [(t',f) rows, (b, i, c, q) cols], n = 8q+i
    xq = x.reshape(B, P, NT, F, T)  # [b, q, i, f, t]
    xt = xq.transpose(0, 2, 4, 3, 1)  # [b, i, t, f, q]
    xt = xt.reshape(B, NT, 2, 6, F, P).reshape(B, NT, 2, 6 * F, P)
    xt = np.ascontiguousarray(xt.transpose(3, 0, 1, 2, 4))  # [96, b, i, c, q]
    xt = xt.reshape(HC, B, 2 * N).astype(NP_BF16)

    th_dev = np.zeros((HC, KHC), np.float32)
    eye6 = np.eye(6, dtype=np.float32)
    for k in range(K):  # stored order [th2 | th1 | th0]
        th_dev[:, (K - 1 - k) * HC : (K - k) * HC] = np.kron(eye6, Theta[k])
    th_dev = th_dev.astype(NP_BF16)

    in_maps = []
    for c0 in range(NCORES):
        b0 = BL * c0
        xth = np.empty((HC, KHC + BL * 2 * N), NP_BF16)
        xth[:, :KHC] = th_dev
        xth[:, KHC:] = xt[:, b0 : b0 + BL].reshape(HC, -1)
        in_maps.append(
            {
                "a": np.ascontiguousarray(a_dev[b0 : b0 + BL]),
                "xth": xth,
            }
        )
    return in_maps


def depermute_out(dev_out):
    """Device out [BL, h, (t',o), n'=(ci,q)] -> [BL, n=8q+ci, o, t=6h+t']."""
    o = np.asarray(dev_out).astype(np.float32)
    o = o.reshape(BL, 2, 6, O, NT, P).transpose(0, 5, 4, 3, 1, 2)
    return np.ascontiguousarray(o.reshape(BL, N, O, T))


def kernel(x, spatial_attention, adj, Theta):
    if "nc" not in _CACHE:
        _CACHE["nc"] = build_nc()
    nc = _CACHE["nc"]

    in_maps = make_in_maps(x, spatial_attention, adj, Theta)
    res = run_bass_kernel_spmd(nc, in_maps, core_ids=list(range(NCORES)))
    return np.concatenate(
        [depermute_out(res.results[c]["out"]) for c in range(NCORES)], axis=0
    )
